# revision 3
# baseline (speedup 1.0000x reference)
"""Trainium2 Bass kernel for nn_Attention_dot3 (dense_transformer).

Reference computation (per batch b, with xf = x.reshape(C, N), N = H*W):
    q  = Wq @ xf + bq                      [CK, N]
    k  = Wk @ xf + bk                      [CK, N]
    v  = Wv @ xf + bv                      [C, N]
    E  = sigmoid(q^T k) / N^2              [N, N]
    out = g * (v @ E) + x,  g = clip(gamma, -1, 1)

Sharding: data-parallel over batch B=8 across the 8 NeuronCores (one batch
image per core); all params replicated.

Per-core dataflow (all matmuls bf16 inputs, fp32 PSUM accumulation):
    - q/k are computed padded to 128 partitions (rows 64..127 zero) so every
      energy matmul is a standard K=128 matmul.
    - vT[n, c] = (x^T @ Wv^T + bv) * (g/N^2) is computed directly in
      transposed layout (n on partitions) so it can serve as lhsT of the
      second matmul; the g/N^2 scale is folded into Wv/bv on the host.
    - The N x N energy matrix is never materialized: E tiles of [128, 1024]
      are produced in PSUM, sigmoided on ScalarE into SBUF (bf16), and
      immediately consumed by the v @ E accumulation matmuls.
    - Final residual: out = acc + x on VectorE, then DMA to DRAM.
"""

import os
from contextlib import ExitStack

import numpy as np

_CACHE = {}

B, C, H, W, K = 8, 256, 64, 64, 4
CK = C // K  # 64
N = H * W  # 4096
P = 128
JW = 1024  # energy tile width (free dim)
NJ = N // JW  # 4
NI = N // P  # 32 row blocks


def _build_program():
    import concourse.bass as bass
    import concourse.mybir as mybir
    import concourse.tile as tile
    from concourse import bacc
    from concourse.bass import ts

    f32 = mybir.dt.float32
    bf16 = mybir.dt.bfloat16

    nc = bacc.Bacc("TRN2", target_bir_lowering=False, debug=False, num_devices=8)

    x_d = nc.dram_tensor("x", [P, 2, N], f32, kind="ExternalInput")
    wq_d = nc.dram_tensor("wqT", [P, 2, CK], bf16, kind="ExternalInput")
    wk_d = nc.dram_tensor("wkT", [P, 2, CK], bf16, kind="ExternalInput")
    wv_d = nc.dram_tensor("wvTs", [P, 2, C], bf16, kind="ExternalInput")
    bq_d = nc.dram_tensor("bq", [CK, 1], f32, kind="ExternalInput")
    bk_d = nc.dram_tensor("bk", [CK, 1], f32, kind="ExternalInput")
    bvb_d = nc.dram_tensor("bvb", [P, C], f32, kind="ExternalInput")
    out_d = nc.dram_tensor("out", [P, 2, N], f32, kind="ExternalOutput")

    ident = mybir.ActivationFunctionType.Identity
    sigm = mybir.ActivationFunctionType.Sigmoid

    with ExitStack() as ctx:
        tc = ctx.enter_context(tile.TileContext(nc))
        consts = ctx.enter_context(tc.tile_pool(name="consts", bufs=1))
        epool = ctx.enter_context(tc.tile_pool(name="epool", bufs=3))
        rpool = ctx.enter_context(tc.tile_pool(name="rpool", bufs=2))
        pse_pool = ctx.enter_context(tc.tile_pool(name="pse", bufs=2, space="PSUM"))
        acc_pool = ctx.enter_context(tc.tile_pool(name="acc", bufs=2, space="PSUM"))

        # ---- constants / weights ----
        wq_sb = consts.tile([P, 2, CK], bf16, name="wq_sb")
        wk_sb = consts.tile([P, 2, CK], bf16, name="wk_sb")
        wv_sb = consts.tile([P, 2, C], bf16, name="wv_sb")
        bq_sb = consts.tile([CK, 1], f32, name="bq_sb")
        bk_sb = consts.tile([CK, 1], f32, name="bk_sb")
        bvb_sb = consts.tile([P, C], f32, name="bvb_sb")
        nc.sync.dma_start(wq_sb[:], wq_d[:])
        nc.sync.dma_start(wk_sb[:], wk_d[:])
        nc.sync.dma_start(wv_sb[:], wv_d[:])
        nc.sync.dma_start(bq_sb[:], bq_d[:])
        nc.sync.dma_start(bk_sb[:], bk_d[:])
        nc.sync.dma_start(bvb_sb[:], bvb_d[:])

        # ---- x load + bf16 convert ----
        x_sb = consts.tile([P, 2, N], f32, name="x_sb")
        xbf = consts.tile([P, 2, N], bf16, name="xbf")
        for o in range(2):
            for h in range(2):
                sl = slice(h * (N // 2), (h + 1) * (N // 2))
                nc.sync.dma_start(x_sb[:, o, sl], x_d[:, o, sl])
                nc.vector.tensor_copy(xbf[:, o, sl], x_sb[:, o, sl])

        # ---- q/k padded tiles and vT ----
        qpad = consts.tile([P, N], bf16, name="qpad")
        kpad = consts.tile([P, N], bf16, name="kpad")
        vt = consts.tile([P, NI, C], bf16, name="vt")
        nc.gpsimd.memset(qpad[CK:P, :], 0.0)
        nc.gpsimd.memset(kpad[CK:P, :], 0.0)

        for w_sb, b_sb, dst in ((wq_sb, bq_sb, qpad), (wk_sb, bk_sb, kpad)):
            for nb in range(N // 512):
                ps = pse_pool.tile([CK, 512], f32, tag="eps", name="ps_qk")
                nc.tensor.matmul(
                    ps[:], w_sb[:, 0, :], xbf[:, 0, ts(nb, 512)], start=True, stop=False
                )
                nc.tensor.matmul(
                    ps[:], w_sb[:, 1, :], xbf[:, 1, ts(nb, 512)], start=False, stop=True
                )
                nc.scalar.activation(dst[0:CK, ts(nb, 512)], ps[:], ident, bias=b_sb[:])

        for ib in range(NI):
            ps = pse_pool.tile([P, C], f32, tag="eps", name="ps_v")
            nc.tensor.matmul(
                ps[:], xbf[:, 0, ts(ib, P)], wv_sb[:, 0, :], start=True, stop=False
            )
            nc.tensor.matmul(
                ps[:], xbf[:, 1, ts(ib, P)], wv_sb[:, 1, :], start=False, stop=True
            )
            nc.vector.tensor_add(vt[:, ib, :], ps[:], bvb_sb[:])

        # ---- main loop: E tiles -> sigmoid -> v @ E accumulation ----
        def e_gen(jb, ib):
            ps = pse_pool.tile([P, JW], f32, tag="eps", name="ps_e")
            for s2 in range(JW // 512):
                nc.tensor.matmul(
                    ps[:, ts(s2, 512)],
                    qpad[:, ts(ib, P)],
                    kpad[:, jb * JW + s2 * 512 : jb * JW + (s2 + 1) * 512],
                    start=True,
                    stop=True,
                )
            return ps

        ps_cur = e_gen(0, 0)
        for jb in range(NJ):
            acc0 = acc_pool.tile([P, JW], f32, tag="acc", name="acc0")
            acc1 = acc_pool.tile([P, JW], f32, tag="acc", name="acc1")
            for ib in range(NI):
                et = epool.tile([P, JW], bf16, tag="et", name="et")
                nc.scalar.activation(et[:], ps_cur[:], sigm)
                if ib + 1 < NI:
                    ps_cur = e_gen(jb, ib + 1)
                elif jb + 1 < NJ:
                    ps_cur = e_gen(jb + 1, 0)
                for acc, cc in ((acc0, 0), (acc1, 1)):
                    for s2 in range(JW // 512):
                        nc.tensor.matmul(
                            acc[:, ts(s2, 512)],
                            vt[:, ib, ts(cc, P)],
                            et[:, ts(s2, 512)],
                            start=(ib == 0),
                            stop=(ib == NI - 1),
                        )
            for acc, cc in ((acc0, 0), (acc1, 1)):
                res = rpool.tile([P, JW], f32, tag="res", name="res")
                nc.vector.tensor_add(res[:], acc[:], x_sb[:, cc, ts(jb, JW)])
                nc.sync.dma_start(out_d[:, cc, ts(jb, JW)], res[:])

    nc.compile()
    return nc


def _prep_inputs(x, Wq, bq, Wk, bk, Wv, bv, gamma):
    import ml_dtypes

    bf16 = ml_dtypes.bfloat16
    g = float(np.clip(np.asarray(gamma, dtype=np.float64), -1.0, 1.0).reshape(()))
    s = g / float(N * N)

    def part(a):  # [C, F...] -> [P, 2, F...] partition-split, contiguous
        a = np.asarray(a)
        return np.ascontiguousarray(a.reshape(2, P, *a.shape[1:]).transpose(1, 0, *range(2, a.ndim + 1)))

    wqT = part(np.asarray(Wq, np.float32).T.astype(bf16))  # [128, 2, 64]
    wkT = part(np.asarray(Wk, np.float32).T.astype(bf16))
    wvTs = part((np.asarray(Wv, np.float32).T * s).astype(bf16))  # [128, 2, 256]
    bq_a = np.ascontiguousarray(np.asarray(bq, np.float32).reshape(CK, 1))
    bk_a = np.ascontiguousarray(np.asarray(bk, np.float32).reshape(CK, 1))
    bvb = np.ascontiguousarray(
        np.tile((np.asarray(bv, np.float32) * s)[None, :], (P, 1)).astype(np.float32)
    )

    x = np.asarray(x, np.float32)
    in_maps = []
    for b in range(B):
        xb = part(x[b].reshape(C, N))  # [128, 2, 4096] f32
        in_maps.append(
            {
                "x": xb,
                "wqT": wqT,
                "wkT": wkT,
                "wvTs": wvTs,
                "bq": bq_a,
                "bk": bk_a,
                "bvb": bvb,
            }
        )
    return in_maps


def _ensure_axon_ntff_hook():
    """The agent image's antenv lacks axon_hooks; bass_utils imports it on the
    trace path. Install a ctypes-backed stand-in (mirrors trn_boot.py)."""
    import contextlib
    import ctypes
    import sys
    import types

    try:
        import antenv.axon_hooks  # noqa: F401

        return
    except ImportError:
        pass

    hook = None
    so_path = "/opt/axon/libaxon_pjrt.so"
    if os.path.exists(so_path):
        lib = ctypes.CDLL(so_path)
        if hasattr(lib, "axon_start_nrt_profile"):
            lib.axon_start_nrt_profile.argtypes = [
                ctypes.POINTER(ctypes.c_int64),
                ctypes.c_size_t,
            ]
            lib.axon_start_nrt_profile.restype = ctypes.c_int64
            lib.axon_stop_nrt_profile.argtypes = [ctypes.c_char_p]
            lib.axon_stop_nrt_profile.restype = ctypes.c_int64

            @contextlib.contextmanager
            def _hook(output_dir, device_ids):
                import jax

                jax.devices()
                if device_ids:
                    ids = (ctypes.c_int64 * len(device_ids))(*device_ids)
                    rc = lib.axon_start_nrt_profile(ids, len(device_ids))
                else:
                    rc = lib.axon_start_nrt_profile(None, 0)
                if rc != 0:
                    raise RuntimeError(f"axon_start_nrt_profile rc={rc}")
                try:
                    yield
                finally:
                    n = lib.axon_stop_nrt_profile(str(output_dir).encode())
                    print(f"profile: {n} file(s) -> {output_dir}", file=sys.stderr)

            hook = _hook

    import antenv

    mod = types.ModuleType("antenv.axon_hooks")
    mod._hook = hook
    mod.get_axon_ntff_profile_hook = lambda: mod._hook

    def set_axon_ntff_profile_hook(h):
        mod._hook = h

    mod.set_axon_ntff_profile_hook = set_axon_ntff_profile_hook
    sys.modules["antenv.axon_hooks"] = mod
    antenv.axon_hooks = mod


def kernel(x, Wq, bq, Wk, bk, Wv, bv, gamma):
    from concourse.bass_utils import run_bass_kernel_spmd

    if "nc" not in _CACHE:
        _CACHE["nc"] = _build_program()
    nc = _CACHE["nc"]

    in_maps = _prep_inputs(x, Wq, bq, Wk, bk, Wv, bv, gamma)
    trace = bool(int(os.environ.get("KERNEL_TRACE", "0")))
    if trace:
        _ensure_axon_ntff_hook()
    br = run_bass_kernel_spmd(
        nc, in_maps, core_ids=list(range(B)), trace=trace
    )
    _CACHE["last_results"] = br

    out = np.empty((B, C, H, W), dtype=np.float32)
    for b in range(B):
        ob = br.results[b]["out"]  # [128, 2, 4096]
        out[b] = ob.transpose(1, 0, 2).reshape(C, N).reshape(C, H, W)
    return out


# revision 5
# speedup vs baseline: 1.0610x; 1.0610x over previous
"""Trainium2 Bass kernel for nn_Attention_dot3 (dense_transformer).

Reference computation (per batch b, with xf = x.reshape(C, N), N = H*W):
    q  = Wq @ xf + bq                      [CK, N]
    k  = Wk @ xf + bk                      [CK, N]
    v  = Wv @ xf + bv                      [C, N]
    E  = sigmoid(q^T k) / N^2              [N, N]
    out = g * (v @ E) + x,  g = clip(gamma, -1, 1)

Sharding: data-parallel over batch B=8 across the 8 NeuronCores (one batch
image per core); all params replicated.

Per-core dataflow (all matmuls bf16 inputs, fp32 PSUM accumulation):
    - q/k are computed padded to 128 partitions (rows 64..127 zero) so every
      energy matmul is a standard K=128 matmul.
    - vT[n, c] = (x^T @ Wv^T + bv) * (g/N^2) is computed directly in
      transposed layout (n on partitions) so it can serve as lhsT of the
      second matmul; the g/N^2 scale is folded into Wv/bv on the host.
    - The N x N energy matrix is never materialized: E tiles of [128, 1024]
      are produced in PSUM, sigmoided on ScalarE into SBUF (bf16), and
      immediately consumed by the v @ E accumulation matmuls.
    - Final residual: out = acc + x on VectorE, then DMA to DRAM.
"""

import os
from contextlib import ExitStack

import numpy as np

_CACHE = {}

B, C, H, W, K = 8, 256, 64, 64, 4
CK = C // K  # 64
N = H * W  # 4096
P = 128
JW = 512  # j-block width (columns of E per accumulation pass)
NJ = N // JW  # 8
NI = N // P  # 32 row blocks
NT = NI // 2  # 16 row-block pairs (two K=64 matmuls packed per PE pass)


def _build_program():
    import concourse.bass as bass
    import concourse.mybir as mybir
    import concourse.tile as tile
    from concourse import bacc
    from concourse.bass import ts

    f32 = mybir.dt.float32
    bf16 = mybir.dt.bfloat16

    nc = bacc.Bacc("TRN2", target_bir_lowering=False, debug=False, num_devices=8)

    x_d = nc.dram_tensor("x", [P, 2, N], f32, kind="ExternalInput")
    wq_d = nc.dram_tensor("wqT", [P, 2, CK], bf16, kind="ExternalInput")
    wk_d = nc.dram_tensor("wkT", [P, 2, CK], bf16, kind="ExternalInput")
    wv_d = nc.dram_tensor("wvTs", [P, 2, C], bf16, kind="ExternalInput")
    bq_d = nc.dram_tensor("bq", [CK, 1], f32, kind="ExternalInput")
    bk_d = nc.dram_tensor("bk", [CK, 1], f32, kind="ExternalInput")
    bvb_d = nc.dram_tensor("bvb", [P, C], f32, kind="ExternalInput")
    out_d = nc.dram_tensor("out", [P, 2, N], f32, kind="ExternalOutput")

    ident = mybir.ActivationFunctionType.Identity
    sigm = mybir.ActivationFunctionType.Sigmoid

    with ExitStack() as ctx:
        tc = ctx.enter_context(tile.TileContext(nc))
        consts = ctx.enter_context(tc.tile_pool(name="consts", bufs=1))
        epool = ctx.enter_context(tc.tile_pool(name="epool", bufs=3))
        rpool = ctx.enter_context(tc.tile_pool(name="rpool", bufs=2))
        pse_pool = ctx.enter_context(tc.tile_pool(name="pse", bufs=2, space="PSUM"))
        acc_pool = ctx.enter_context(tc.tile_pool(name="acc", bufs=4, space="PSUM"))

        # ---- constants / weights ----
        wq_sb = consts.tile([P, 2, CK], bf16, name="wq_sb")
        wk_sb = consts.tile([P, 2, CK], bf16, name="wk_sb")
        wv_sb = consts.tile([P, 2, C], bf16, name="wv_sb")
        bq_sb = consts.tile([CK, 1], f32, name="bq_sb")
        bk_sb = consts.tile([CK, 1], f32, name="bk_sb")
        bvb_sb = consts.tile([P, C], f32, name="bvb_sb")
        nc.sync.dma_start(wq_sb[:], wq_d[:])
        nc.sync.dma_start(wk_sb[:], wk_d[:])
        nc.sync.dma_start(wv_sb[:], wv_d[:])
        nc.sync.dma_start(bq_sb[:], bq_d[:])
        nc.sync.dma_start(bk_sb[:], bk_d[:])
        nc.sync.dma_start(bvb_sb[:], bvb_d[:])

        # ---- streamed startup: load/convert x and build q/k/vT per 512-block ----
        # qdup/kdup hold the CK=64 rows duplicated on partitions 64..127 so two
        # energy matmuls (row-block pair) run concurrently in distinct PE
        # row-groups via tile_position.
        x_sb = consts.tile([P, 2, N], f32, name="x_sb")
        xbf = consts.tile([P, 2, N], bf16, name="xbf")
        qdup = consts.tile([P, N], bf16, name="qdup")
        kdup = consts.tile([P, N], bf16, name="kdup")
        vt = consts.tile([P, NI, C], bf16, name="vt")

        for h in range(N // 512):
            blk = ts(h, 512)
            for o in range(2):
                nc.sync.dma_start(x_sb[:, o, blk], x_d[:, o, blk])
                nc.vector.tensor_copy(xbf[:, o, blk], x_sb[:, o, blk])
            for w_sb, b_sb, dst in ((wq_sb, bq_sb, qdup), (wk_sb, bk_sb, kdup)):
                ps = pse_pool.tile([CK, 512], f32, tag="eps", name="ps_qk")
                nc.tensor.matmul(
                    ps[:], w_sb[:, 0, :], xbf[:, 0, blk], start=True, stop=False
                )
                nc.tensor.matmul(
                    ps[:], w_sb[:, 1, :], xbf[:, 1, blk], start=False, stop=True
                )
                nc.vector.tensor_scalar_add(dst[0:CK, blk], ps[:], b_sb[:])
                nc.sync.dma_start(dst[CK:P, blk], dst[0:CK, blk])
            for ib in range(h * 4, h * 4 + 4):
                ps = pse_pool.tile([P, C], f32, tag="eps", name="ps_v")
                nc.tensor.matmul(
                    ps[:], xbf[:, 0, ts(ib, P)], wv_sb[:, 0, :], start=True, stop=False
                )
                nc.tensor.matmul(
                    ps[:], xbf[:, 1, ts(ib, P)], wv_sb[:, 1, :], start=False, stop=True
                )
                nc.vector.tensor_add(vt[:, ib, :], ps[:], bvb_sb[:])

        # ---- main loop: paired E tiles -> sigmoid -> v @ E accumulation ----
        # One "super tile" = [128, 1024] PSUM holding E(i0, j-block) | E(i1,
        # j-block), produced by two concurrent K=64 matmuls in row-groups 0/1.
        def e_gen(j, t):
            ps = pse_pool.tile([P, 2 * JW], f32, tag="eps", name="ps_e")
            for tt in range(2):
                i = 2 * t + tt
                nc.tensor.matmul(
                    ps[:, ts(tt, JW)],
                    qdup[tt * CK : (tt + 1) * CK, ts(i, P)],
                    kdup[tt * CK : (tt + 1) * CK, ts(j, JW)],
                    start=True,
                    stop=True,
                    tile_position=(tt * CK, 0),
                )
            return ps

        ps_cur = e_gen(0, 0)
        for j in range(NJ):
            acc0 = acc_pool.tile([P, JW], f32, tag="acc", name="acc0")
            acc1 = acc_pool.tile([P, JW], f32, tag="acc", name="acc1")
            for t in range(NT):
                et = epool.tile([P, 2 * JW], bf16, tag="et", name="et")
                nc.scalar.activation(et[:], ps_cur[:], sigm)
                if t + 1 < NT:
                    ps_cur = e_gen(j, t + 1)
                elif j + 1 < NJ:
                    ps_cur = e_gen(j + 1, 0)
                for tt in range(2):
                    i = 2 * t + tt
                    for acc, cc in ((acc0, 0), (acc1, 1)):
                        nc.tensor.matmul(
                            acc[:],
                            vt[:, i, ts(cc, P)],
                            et[:, ts(tt, JW)],
                            start=(t == 0 and tt == 0),
                            stop=(t == NT - 1 and tt == 1),
                        )
            for acc, cc in ((acc0, 0), (acc1, 1)):
                res = rpool.tile([P, JW], f32, tag="res", name="res")
                nc.vector.tensor_add(res[:], acc[:], x_sb[:, cc, ts(j, JW)])
                nc.sync.dma_start(out_d[:, cc, ts(j, JW)], res[:])

    nc.compile()
    return nc


def _prep_inputs(x, Wq, bq, Wk, bk, Wv, bv, gamma):
    import ml_dtypes

    bf16 = ml_dtypes.bfloat16
    g = float(np.clip(np.asarray(gamma, dtype=np.float64), -1.0, 1.0).reshape(()))
    s = g / float(N * N)

    def part(a):  # [C, F...] -> [P, 2, F...] partition-split, contiguous
        a = np.asarray(a)
        return np.ascontiguousarray(a.reshape(2, P, *a.shape[1:]).transpose(1, 0, *range(2, a.ndim + 1)))

    wqT = part(np.asarray(Wq, np.float32).T.astype(bf16))  # [128, 2, 64]
    wkT = part(np.asarray(Wk, np.float32).T.astype(bf16))
    wvTs = part((np.asarray(Wv, np.float32).T * s).astype(bf16))  # [128, 2, 256]
    bq_a = np.ascontiguousarray(np.asarray(bq, np.float32).reshape(CK, 1))
    bk_a = np.ascontiguousarray(np.asarray(bk, np.float32).reshape(CK, 1))
    bvb = np.ascontiguousarray(
        np.tile((np.asarray(bv, np.float32) * s)[None, :], (P, 1)).astype(np.float32)
    )

    x = np.asarray(x, np.float32)
    in_maps = []
    for b in range(B):
        xb = part(x[b].reshape(C, N))  # [128, 2, 4096] f32
        in_maps.append(
            {
                "x": xb,
                "wqT": wqT,
                "wkT": wkT,
                "wvTs": wvTs,
                "bq": bq_a,
                "bk": bk_a,
                "bvb": bvb,
            }
        )
    return in_maps


def _ensure_axon_ntff_hook():
    """The agent image's antenv lacks axon_hooks; bass_utils imports it on the
    trace path. Install a ctypes-backed stand-in (mirrors trn_boot.py)."""
    import contextlib
    import ctypes
    import sys
    import types

    try:
        import antenv.axon_hooks  # noqa: F401

        return
    except ImportError:
        pass

    hook = None
    so_path = "/opt/axon/libaxon_pjrt.so"
    if os.path.exists(so_path):
        lib = ctypes.CDLL(so_path)
        if hasattr(lib, "axon_start_nrt_profile"):
            lib.axon_start_nrt_profile.argtypes = [
                ctypes.POINTER(ctypes.c_int64),
                ctypes.c_size_t,
            ]
            lib.axon_start_nrt_profile.restype = ctypes.c_int64
            lib.axon_stop_nrt_profile.argtypes = [ctypes.c_char_p]
            lib.axon_stop_nrt_profile.restype = ctypes.c_int64

            @contextlib.contextmanager
            def _hook(output_dir, device_ids):
                import jax

                jax.devices()
                if device_ids:
                    ids = (ctypes.c_int64 * len(device_ids))(*device_ids)
                    rc = lib.axon_start_nrt_profile(ids, len(device_ids))
                else:
                    rc = lib.axon_start_nrt_profile(None, 0)
                if rc != 0:
                    raise RuntimeError(f"axon_start_nrt_profile rc={rc}")
                try:
                    yield
                finally:
                    n = lib.axon_stop_nrt_profile(str(output_dir).encode())
                    print(f"profile: {n} file(s) -> {output_dir}", file=sys.stderr)

            hook = _hook

    import antenv

    mod = types.ModuleType("antenv.axon_hooks")
    mod._hook = hook
    mod.get_axon_ntff_profile_hook = lambda: mod._hook

    def set_axon_ntff_profile_hook(h):
        mod._hook = h

    mod.set_axon_ntff_profile_hook = set_axon_ntff_profile_hook
    sys.modules["antenv.axon_hooks"] = mod
    antenv.axon_hooks = mod


def kernel(x, Wq, bq, Wk, bk, Wv, bv, gamma):
    from concourse.bass_utils import run_bass_kernel_spmd

    if "nc" not in _CACHE:
        _CACHE["nc"] = _build_program()
    nc = _CACHE["nc"]

    in_maps = _prep_inputs(x, Wq, bq, Wk, bk, Wv, bv, gamma)
    trace = bool(int(os.environ.get("KERNEL_TRACE", "0")))
    if trace:
        _ensure_axon_ntff_hook()
    br = run_bass_kernel_spmd(
        nc, in_maps, core_ids=list(range(B)), trace=trace
    )
    _CACHE["last_results"] = br

    out = np.empty((B, C, H, W), dtype=np.float32)
    for b in range(B):
        ob = br.results[b]["out"]  # [128, 2, 4096]
        out[b] = ob.transpose(1, 0, 2).reshape(C, N).reshape(C, H, W)
    return out


# revision 10
# speedup vs baseline: 1.1479x; 1.0819x over previous
"""Trainium2 Bass kernel for nn_Attention_dot3 (dense_transformer).

Reference computation (per batch b, with xf = x.reshape(C, N), N = H*W):
    q  = Wq @ xf + bq                      [CK, N]
    k  = Wk @ xf + bk                      [CK, N]
    v  = Wv @ xf + bv                      [C, N]
    E  = sigmoid(q^T k) / N^2              [N, N]
    out = g * (v @ E) + x,  g = clip(gamma, -1, 1)

Sharding: data-parallel over batch B=8 across the 8 NeuronCores (one batch
image per core); all params replicated.

Per-core dataflow:
    - q/k are computed into [128, N] tiles with the CK=64 rows duplicated on
      partitions 64..127, so pairs of energy matmuls run CONCURRENTLY in the
      two 64-row PE row-groups (tile_position packing for K=64).
    - vT[n, c] = x^T @ Wv^T + bv is computed directly in transposed layout
      (n on partitions) in fp8e4 so it can serve as the DoubleRow stationary
      operand of the second matmul. v values are O(1) so fp8e4 range is fine;
      the g/N^2 scale is applied at the final residual.
    - The N x N energy matrix is never materialized: "super tiles" holding
      E(i0,j)|E(i1,j) of [128, 2*512] are produced in PSUM by the concurrent
      matmul pair, sigmoided on ScalarE into SBUF as fp8e4, and immediately
      consumed by fp8 DoubleRow v @ E matmuls (contraction 256 per matmul).
    - Startup (x load, q/k/vT generation) is software-pipelined into the
      first j-pass of the main loop.
    - Final residual: out = acc * (g/N^2) + x fused on VectorE, DMA out.
"""

import os
from contextlib import ExitStack

import numpy as np

_CACHE = {}

B, C, H, W, K = 8, 256, 64, 64, 4
CK = C // K  # 64
N = H * W  # 4096
P = 128
JW = 512  # j-block width (columns of E per accumulation pass)
NJ = N // JW  # 8
NI = N // P  # 32 row blocks
NT = NI // 2  # 16 row-block pairs
NH = N // 512  # 8 column blocks for the generation phase
SCALE = 1.0 / float(N * N)


def _build_program():
    import concourse.bass as bass
    import concourse.mybir as mybir
    import concourse.tile as tile
    from concourse import bacc
    from concourse.bass import ts

    f32 = mybir.dt.float32
    bf16 = mybir.dt.bfloat16
    f8 = mybir.dt.float8e4

    nc = bacc.Bacc("TRN2", target_bir_lowering=False, debug=False, num_devices=8)

    x_d = nc.dram_tensor("x", [P, 2, N], f32, kind="ExternalInput")
    xb_d = nc.dram_tensor("xb", [P, 2, N], bf16, kind="ExternalInput")
    wq_d = nc.dram_tensor("wqT", [P, 2, CK], bf16, kind="ExternalInput")
    wk_d = nc.dram_tensor("wkT", [P, 2, CK], bf16, kind="ExternalInput")
    wv_d = nc.dram_tensor("wvT", [P, 2, C], bf16, kind="ExternalInput")
    bq_d = nc.dram_tensor("bq", [CK, 1], f32, kind="ExternalInput")
    bk_d = nc.dram_tensor("bk", [CK, 1], f32, kind="ExternalInput")
    bvb_d = nc.dram_tensor("bvb", [P, C], f32, kind="ExternalInput")
    gs_d = nc.dram_tensor("gscale", [P, 1], f32, kind="ExternalInput")
    out_d = nc.dram_tensor("out", [P, 2, N], f32, kind="ExternalOutput")

    sigm = mybir.ActivationFunctionType.Sigmoid

    with ExitStack() as ctx:
        tc = ctx.enter_context(tile.TileContext(nc))
        consts = ctx.enter_context(tc.tile_pool(name="consts", bufs=1))
        epool = ctx.enter_context(tc.tile_pool(name="epool", bufs=3))
        rpool = ctx.enter_context(tc.tile_pool(name="rpool", bufs=2))
        pse_pool = ctx.enter_context(tc.tile_pool(name="pse", bufs=2, space="PSUM"))
        acc_pool = ctx.enter_context(tc.tile_pool(name="acc", bufs=4, space="PSUM"))

        # ---- constants / weights (sync queue) ----
        wq_sb = consts.tile([P, 2, CK], bf16, name="wq_sb")
        wk_sb = consts.tile([P, 2, CK], bf16, name="wk_sb")
        wv_sb = consts.tile([P, 2, C], bf16, name="wv_sb")
        bq_sb = consts.tile([CK, 1], f32, name="bq_sb")
        bk_sb = consts.tile([CK, 1], f32, name="bk_sb")
        bvb_sb = consts.tile([P, C], f32, name="bvb_sb")
        gs_sb = consts.tile([P, 1], f32, name="gs_sb")
        nc.sync.dma_start(wq_sb[:], wq_d[:])
        nc.sync.dma_start(wk_sb[:], wk_d[:])
        nc.sync.dma_start(wv_sb[:], wv_d[:])
        nc.sync.dma_start(bq_sb[:], bq_d[:])
        nc.sync.dma_start(bk_sb[:], bk_d[:])
        nc.sync.dma_start(bvb_sb[:], bvb_d[:])
        nc.sync.dma_start(gs_sb[:], gs_d[:])

        x_sb = consts.tile([P, 2, N], f32, name="x_sb")
        xbf = consts.tile([P, 2, N], bf16, name="xbf")
        qdup = consts.tile([P, N], bf16, name="qdup")
        kdup = consts.tile([P, N], bf16, name="kdup")
        vt = consts.tile([P, NI, C], f8, name="vt")

        # one generation block: load x columns [512h, 512h+512), build q/k/vT
        def gen_block(h):
            blk = ts(h, 512)
            nc.gpsimd.dma_start(x_sb[:, 0, blk], x_d[:, 0, blk])
            nc.gpsimd.dma_start(x_sb[:, 1, blk], x_d[:, 1, blk])
            nc.sync.dma_start(xbf[:, 0, blk], xb_d[:, 0, blk])
            nc.sync.dma_start(xbf[:, 1, blk], xb_d[:, 1, blk])
            for w_sb, b_sb, dst in ((wq_sb, bq_sb, qdup), (wk_sb, bk_sb, kdup)):
                ps = pse_pool.tile([CK, 512], f32, tag="eps", name="ps_qk")
                nc.tensor.matmul(
                    ps[:], w_sb[:, 0, :], xbf[:, 0, blk], start=True, stop=False
                )
                nc.tensor.matmul(
                    ps[:], w_sb[:, 1, :], xbf[:, 1, blk], start=False, stop=True
                )
                nc.vector.tensor_scalar_add(dst[0:CK, blk], ps[:], b_sb[:])
                nc.sync.dma_start(dst[CK:P, blk], dst[0:CK, blk])
            for ib in range(h * 4, h * 4 + 4):
                ps = pse_pool.tile([P, C], f32, tag="eps", name="ps_v")
                nc.tensor.matmul(
                    ps[:], xbf[:, 0, ts(ib, P)], wv_sb[:, 0, :], start=True, stop=False
                )
                nc.tensor.matmul(
                    ps[:], xbf[:, 1, ts(ib, P)], wv_sb[:, 1, :], start=False, stop=True
                )
                nc.vector.tensor_add(vt[:, ib, :], ps[:], bvb_sb[:])

        # ---- main loop ----
        def e_gen(j, t):
            ps = pse_pool.tile([P, 2 * JW], f32, tag="eps", name="ps_e")
            for tt in range(2):
                i = 2 * t + tt
                nc.tensor.matmul(
                    ps[:, ts(tt, JW)],
                    qdup[tt * CK : (tt + 1) * CK, ts(i, P)],
                    kdup[tt * CK : (tt + 1) * CK, ts(j, JW)],
                    start=True,
                    stop=True,
                    tile_position=(tt * CK, 0),
                )
            return ps

        gen_block(0)
        ps_cur = e_gen(0, 0)
        for j in range(NJ):
            acc0 = acc_pool.tile([P, JW], f32, tag="acc", name="acc0")
            acc1 = acc_pool.tile([P, JW], f32, tag="acc", name="acc1")
            for t in range(NT):
                # interleave remaining generation blocks into the first j-pass
                if j == 0 and t % 2 == 0 and t // 2 + 1 < NH:
                    gen_block(t // 2 + 1)
                et = epool.tile([P, 2, JW], f8, tag="et", name="et")
                nc.scalar.activation(et.rearrange("p a b -> p (a b)"), ps_cur[:], sigm)
                if t + 1 < NT:
                    ps_cur = e_gen(j, t + 1)
                elif j + 1 < NJ:
                    ps_cur = e_gen(j + 1, 0)
                for acc, cc in ((acc0, 0), (acc1, 1)):
                    nc.tensor.matmul(
                        acc[:],
                        vt[:, 2 * t : 2 * t + 2, ts(cc, P)],
                        et[:],
                        start=(t == 0),
                        stop=(t == NT - 1),
                        perf_mode=mybir.MatmulPerfMode.DoubleRow,
                    )
            for acc, cc in ((acc0, 0), (acc1, 1)):
                res = rpool.tile([P, JW], f32, tag="res", name="res")
                nc.vector.scalar_tensor_tensor(
                    res[:],
                    acc[:],
                    gs_sb[:],
                    x_sb[:, cc, ts(j, JW)],
                    mybir.AluOpType.mult,
                    mybir.AluOpType.add,
                )
                nc.sync.dma_start(out_d[:, cc, ts(j, JW)], res[:])

    nc.compile()
    return nc


def _prep_inputs(x, Wq, bq, Wk, bk, Wv, bv, gamma):
    import ml_dtypes

    bf16 = ml_dtypes.bfloat16
    g = float(np.clip(np.asarray(gamma, dtype=np.float64), -1.0, 1.0).reshape(()))

    def part(a):  # [C, F...] -> [P, 2, F...] partition-split, contiguous
        a = np.asarray(a)
        return np.ascontiguousarray(
            a.reshape(2, P, *a.shape[1:]).transpose(1, 0, *range(2, a.ndim + 1))
        )

    wqT = part(np.asarray(Wq, np.float32).T.astype(bf16))  # [128, 2, 64]
    wkT = part(np.asarray(Wk, np.float32).T.astype(bf16))
    wvT = part(np.asarray(Wv, np.float32).T.astype(bf16))  # [128, 2, 256]
    bq_a = np.ascontiguousarray(np.asarray(bq, np.float32).reshape(CK, 1))
    bk_a = np.ascontiguousarray(np.asarray(bk, np.float32).reshape(CK, 1))
    bvb = np.ascontiguousarray(
        np.tile(np.asarray(bv, np.float32)[None, :], (P, 1)).astype(np.float32)
    )
    gs = np.full((P, 1), g * SCALE, dtype=np.float32)

    x = np.asarray(x, np.float32)
    in_maps = []
    for b in range(B):
        xb = part(x[b].reshape(C, N))  # [128, 2, 4096] f32
        in_maps.append(
            {
                "x": xb,
                "xb": xb.astype(bf16),
                "wqT": wqT,
                "wkT": wkT,
                "wvT": wvT,
                "bq": bq_a,
                "bk": bk_a,
                "bvb": bvb,
                "gscale": gs,
            }
        )
    return in_maps


def _ensure_axon_ntff_hook():
    """The agent image's antenv lacks axon_hooks; bass_utils imports it on the
    trace path. Install a ctypes-backed stand-in (mirrors trn_boot.py)."""
    import contextlib
    import ctypes
    import sys
    import types

    try:
        import antenv.axon_hooks  # noqa: F401

        return
    except ImportError:
        pass

    hook = None
    so_path = "/opt/axon/libaxon_pjrt.so"
    if os.path.exists(so_path):
        lib = ctypes.CDLL(so_path)
        if hasattr(lib, "axon_start_nrt_profile"):
            lib.axon_start_nrt_profile.argtypes = [
                ctypes.POINTER(ctypes.c_int64),
                ctypes.c_size_t,
            ]
            lib.axon_start_nrt_profile.restype = ctypes.c_int64
            lib.axon_stop_nrt_profile.argtypes = [ctypes.c_char_p]
            lib.axon_stop_nrt_profile.restype = ctypes.c_int64

            @contextlib.contextmanager
            def _hook(output_dir, device_ids):
                import jax

                jax.devices()
                if device_ids:
                    ids = (ctypes.c_int64 * len(device_ids))(*device_ids)
                    rc = lib.axon_start_nrt_profile(ids, len(device_ids))
                else:
                    rc = lib.axon_start_nrt_profile(None, 0)
                if rc != 0:
                    raise RuntimeError(f"axon_start_nrt_profile rc={rc}")
                try:
                    yield
                finally:
                    n = lib.axon_stop_nrt_profile(str(output_dir).encode())
                    print(f"profile: {n} file(s) -> {output_dir}", file=sys.stderr)

            hook = _hook

    import antenv

    mod = types.ModuleType("antenv.axon_hooks")
    mod._hook = hook
    mod.get_axon_ntff_profile_hook = lambda: mod._hook

    def set_axon_ntff_profile_hook(h):
        mod._hook = h

    mod.set_axon_ntff_profile_hook = set_axon_ntff_profile_hook
    sys.modules["antenv.axon_hooks"] = mod
    antenv.axon_hooks = mod


def kernel(x, Wq, bq, Wk, bk, Wv, bv, gamma):
    from concourse.bass_utils import run_bass_kernel_spmd

    if "nc" not in _CACHE:
        _CACHE["nc"] = _build_program()
    nc = _CACHE["nc"]

    in_maps = _prep_inputs(x, Wq, bq, Wk, bk, Wv, bv, gamma)
    trace = bool(int(os.environ.get("KERNEL_TRACE", "0")))
    if trace:
        _ensure_axon_ntff_hook()
    br = run_bass_kernel_spmd(nc, in_maps, core_ids=list(range(B)), trace=trace)
    _CACHE["last_results"] = br

    out = np.empty((B, C, H, W), dtype=np.float32)
    for b in range(B):
        ob = br.results[b]["out"]  # [128, 2, 4096]
        out[b] = ob.transpose(1, 0, 2).reshape(C, N).reshape(C, H, W)
    return out
